# revision 5
# baseline (speedup 1.0000x reference)
"""Cosine-similarity attention on 8 Trainium2 NeuronCores.

reference:
    attn = l2norm(keys) @ l2norm(queries).T          # [Nk, Nq]
    weighted = attn.T @ values                        # [Nq, D]
    returns (weighted, attn)

Sharding: queries split row-wise across 8 cores (512 rows each); keys/values
replicated. Core i computes attn[:, 512i:512i+512] and weighted[512i:512i+512, :].

Per-core dataflow (Nq_shard=512, Nk=4096, D=1024), all matmuls in fp32r
(TensorE reduced-precision fp32, 1 cycle/row at N>=512):
  - Q: load, row-ssq (ACT Square+accum), rinv=1/sqrt, scale rows (DVE),
    PE-transpose -> QT[d][128, 512] (8 tiles, d-major).
  - per K tile kt (32 x [128, 1024]): load raw, row-ssq, PE-transpose raw K
    -> KT [128(d), 1024(=8 d-blocks x 128 k)], 8 accumulating matmuls
    psum_attn += KT[db].T @ QT[db]  -> S tile [128(k), 512(q)],
    attn_sb[kt] = psum_attn * rinv_k (scaled copy, ACT) -> store to HBM.
  - weighted^T: for kt: for db2 in 8: psum_w[db2] += V[kt][:, db2].T @ attn_sb[kt]
    (V in natural layout as stationary operand; no V transpose needed).
    Then per db2: copy psum->sbuf, 4 PE transposes back to [q, d] layout,
    assemble weighted rows, store.
"""
import sys
import numpy as np

sys.path.insert(0, "/opt/trn_rl_repo")

import concourse.bass as bass  # noqa: E402
import concourse.bacc as bacc  # noqa: E402
import concourse.mybir as mybir  # noqa: E402
import concourse.tile as tile  # noqa: E402
from concourse.bass_utils import run_bass_kernel_spmd  # noqa: E402
from concourse.masks import make_identity  # noqa: E402
from contextlib import ExitStack  # noqa: E402

F32 = mybir.dt.float32
F32R = mybir.dt.float32r

NQ, NK, D = 4096, 4096, 1024
NCORES = 8
MQ = NQ // NCORES          # 512 query rows per core
NKT = NK // 128            # 32 key tiles
NDB = D // 128             # 8 d-blocks
NQT = MQ // 128            # 4 q tiles
KC = 2                     # K/V DMA chunk, in 128-row tiles (1 MB per chunk)


def build_nc():
    nc = bacc.Bacc()
    q = nc.declare_dram_parameter("q", [MQ, D], F32R, isOutput=False)
    keys = nc.declare_dram_parameter("keys", [NK, D], F32R, isOutput=False)
    values = nc.declare_dram_parameter("values", [NK, D], F32R, isOutput=False)
    attn_out = nc.declare_dram_parameter("attn_out", [NK, MQ], F32R, isOutput=True)
    w_out = nc.declare_dram_parameter("w_out", [MQ, D], F32, isOutput=True)

    q_v = q.rearrange("(t p) d -> p t d", p=128)            # [128, 4, 1024]
    keys_v = keys.rearrange("(c t p) d -> c p t d", p=128, t=KC)
    vals_v = values.rearrange("(c t p) d -> c p t d", p=128, t=KC)
    w_v = w_out.rearrange("(j p) d -> p j d", p=128)        # [128, 4, 1024]

    with tile.TileContext(nc) as tc, ExitStack() as ctx:
        const_pool = ctx.enter_context(tc.tile_pool(name="const", bufs=1))
        qt_pool = ctx.enter_context(tc.tile_pool(name="qt", bufs=1))
        attn_pool = ctx.enter_context(tc.tile_pool(name="attn", bufs=1))
        kio_pool = ctx.enter_context(tc.tile_pool(name="kio", bufs=3))
        vio_pool = ctx.enter_context(tc.tile_pool(name="vio", bufs=4))
        work_pool = ctx.enter_context(tc.tile_pool(name="work", bufs=2))
        small_pool = ctx.enter_context(tc.tile_pool(name="small", bufs=4))

        ident_f = const_pool.tile([128, 128], F32)
        make_identity(nc, ident_f[:])
        ident_r = const_pool.tile([128, 128], F32R)
        nc.vector.tensor_copy(ident_r[:], ident_f[:])

        QT = [qt_pool.tile([128, MQ], F32R, name=f"qt{db}") for db in range(NDB)]
        attn_sb = [attn_pool.tile([128, MQ], F32R, name=f"attn{kt}") for kt in range(NKT)]

        # ---------------- Q prep ----------------
        with (
            tc.tile_pool(name="qprep", bufs=1) as qprep_pool,
            tc.tile_pool(name="qps", bufs=2, space="PSUM") as qps_pool,
        ):
            q_io = qprep_pool.tile([128, NQT, D], F32R)
            nc.sync.dma_start(q_io[:], q_v)
            qn = []
            for j in range(NQT):
                ssq = small_pool.tile([128, 1], F32, name=f"qssq{j}", tag="qssq")
                scratch = work_pool.tile([128, D], F32, name=f"qsc{j}", tag="qsc")
                nc.scalar.activation(scratch[:], q_io[:, j, :],
                                     mybir.ActivationFunctionType.Square,
                                     accum_out=ssq[:])
                rinv = small_pool.tile([128, 1], F32, name=f"qrinv{j}", tag="qrinv")
                nc.vector.reciprocal(rinv[:], ssq[:])
                nc.scalar.activation(rinv[:], rinv[:],
                                     mybir.ActivationFunctionType.Sqrt)
                qn_j = qprep_pool.tile([128, D], F32R, name=f"qn{j}")
                nc.vector.tensor_scalar_mul(qn_j[:], q_io[:, j, :], rinv[:])
                qn.append(qn_j)
            for db in range(NDB):
                qt_ps = qps_pool.tile([128, MQ], F32R, name=f"qtps{db}", tag="qtps")
                for j in range(NQT):
                    nc.tensor.transpose(qt_ps[:, j * 128:(j + 1) * 128],
                                        qn[j][:, db * 128:(db + 1) * 128],
                                        ident_r[:])
                nc.vector.tensor_copy(QT[db][:], qt_ps[:])

        # ---------------- phase 1: attn tiles ----------------
        with (
            tc.tile_pool(name="ktps", bufs=2, space="PSUM") as ktps_pool,
            tc.tile_pool(name="aps", bufs=2, space="PSUM") as aps_pool,
        ):
            for c in range(NK // (128 * KC)):
                kc_t = kio_pool.tile([128, KC, D], F32R, name=f"kc{c}", tag="kio")
                nc.sync.dma_start(kc_t[:], keys_v[c])
                for t in range(KC):
                    kt = c * KC + t
                    k_tile = kc_t[:, t, :]
                    ssq = small_pool.tile([128, 1], F32, name=f"kssq{kt}", tag="kssq")
                    scratch = work_pool.tile([128, D], F32, name=f"ksc{kt}", tag="ksc")
                    nc.scalar.activation(scratch[:], k_tile,
                                         mybir.ActivationFunctionType.Square,
                                         accum_out=ssq[:])
                    rinv = small_pool.tile([128, 1], F32, name=f"krinv{kt}", tag="krinv")
                    nc.vector.reciprocal(rinv[:], ssq[:])
                    nc.scalar.activation(rinv[:], rinv[:],
                                         mybir.ActivationFunctionType.Sqrt)

                    kt_ps = ktps_pool.tile([128, D], F32R, name=f"ktps{kt}", tag="ktps")
                    for db in range(NDB):
                        nc.tensor.transpose(kt_ps[:, db * 128:(db + 1) * 128],
                                            k_tile[:, db * 128:(db + 1) * 128],
                                            ident_r[:])
                    kt_sb = work_pool.tile([128, D], F32R, name=f"ktsb{kt}", tag="ktsb")
                    nc.vector.tensor_copy(kt_sb[:], kt_ps[:])

                    a_ps = aps_pool.tile([128, MQ], F32, name=f"aps{kt}", tag="aps")
                    for db in range(NDB):
                        nc.tensor.matmul(a_ps[:],
                                         kt_sb[:, db * 128:(db + 1) * 128],
                                         QT[db][:],
                                         start=(db == 0), stop=(db == NDB - 1))
                    # attn = S * (1/|k|) with cast fp32 -> fp32r, on ACT
                    nc.scalar.activation(attn_sb[kt][:], a_ps[:],
                                         mybir.ActivationFunctionType.Copy,
                                         scale=rinv[:])
                    nc.sync.dma_start(attn_out[kt * 128:(kt + 1) * 128, :],
                                      attn_sb[kt][:])

        # ---------------- phase 2: weighted^T then transpose back ----------------
        wsb_pool = ctx.enter_context(tc.tile_pool(name="wsb", bufs=1))
        with tc.tile_pool(name="wps", bufs=1, space="PSUM") as wps_pool:
            w_ps = [wps_pool.tile([128, MQ], F32, name=f"wps{db}") for db in range(NDB)]
            for c in range(NK // (128 * KC)):
                vc_t = vio_pool.tile([128, KC, D], F32R, name=f"vc{c}", tag="vio")
                nc.sync.dma_start(vc_t[:], vals_v[c])
                for t in range(KC):
                    kt = c * KC + t
                    for db in range(NDB):
                        nc.tensor.matmul(w_ps[db][:],
                                         vc_t[:, t, db * 128:(db + 1) * 128],
                                         attn_sb[kt][:],
                                         start=(kt == 0), stop=(kt == NKT - 1))
            wt_sb = [wsb_pool.tile([128, MQ], F32, name=f"wtsb{db}")
                     for db in range(NDB)]
            for db in range(NDB):
                nc.vector.tensor_copy(wt_sb[db][:], w_ps[db][:])

        w_sb = ctx.enter_context(tc.tile_pool(name="wfin", bufs=1)).tile(
            [128, NQT, D], F32)
        with tc.tile_pool(name="tps", bufs=2, space="PSUM") as tps_pool:
            for db in range(NDB):
                t_ps = tps_pool.tile([128, MQ], F32, name=f"tps{db}", tag="tps")
                for j in range(NQT):
                    nc.tensor.transpose(t_ps[:, j * 128:(j + 1) * 128],
                                        wt_sb[db][:, j * 128:(j + 1) * 128],
                                        ident_f[:])
                nc.vector.tensor_copy(
                    w_sb[:, :, db * 128:(db + 1) * 128],
                    t_ps[:].rearrange("p (j n) -> p j n", n=128))
        nc.sync.dma_start(w_v, w_sb[:])

    nc.finalize()
    return nc


_NC_CACHE = None


def _get_nc():
    global _NC_CACHE
    if _NC_CACHE is None:
        _NC_CACHE = build_nc()
    return _NC_CACHE


def kernel(queries: np.ndarray, keys: np.ndarray, values: np.ndarray):
    queries = np.ascontiguousarray(queries, dtype=np.float32)
    keys = np.ascontiguousarray(keys, dtype=np.float32)
    values = np.ascontiguousarray(values, dtype=np.float32)

    nc = _get_nc()
    in_maps = [
        {"q": queries[i * MQ:(i + 1) * MQ], "keys": keys, "values": values}
        for i in range(NCORES)
    ]
    res = run_bass_kernel_spmd(nc, in_maps, list(range(NCORES)))
    attn = np.concatenate([r["attn_out"] for r in res.results], axis=1)
    weighted = np.concatenate([r["w_out"] for r in res.results], axis=0)
    return (weighted.astype(np.float32, copy=False),
            attn.astype(np.float32, copy=False))


# revision 8
# speedup vs baseline: 1.0402x; 1.0402x over previous
"""Cosine-similarity attention on 8 Trainium2 NeuronCores.

reference:
    attn = l2norm(keys) @ l2norm(queries).T          # [Nk, Nq]
    weighted = attn.T @ values                        # [Nq, D]
    returns (weighted, attn)

Sharding: queries split row-wise across 8 cores (512 rows each); keys/values
replicated. Core i computes attn[:, 512i:512i+512] and weighted[512i:512i+512, :].

Per-core dataflow (Nq_shard=512, Nk=4096, D=1024), matmuls in fp32r
(TensorE reduced-precision fp32, 1 cycle/row at N>=512):
  - Q: load, row-ssq (ACT Square+accum), rinv=1/sqrt, scale rows (DVE),
    PE-transpose -> QT[d][128, 512] (8 tiles, d-major).
  - per K tile kt (32 x [128, 1024]): load raw, row-ssq, PE-transpose raw K
    -> KT [128(d), 1024(=8 d-blocks x 128 k)], 8 accumulating matmuls
    psum_attn += KT[db].T @ QT[db]  -> S tile [128(k), 512(q)],
    attn_sb[kt] = psum_attn * rinv_k (scaled copy with f32r cast, on ACT),
    store to HBM.
  - weighted (natural orientation, both operands in natural layout):
    for kt: for j in 4: for dh in 2:
      psum_w[j][dh] += attn_sb[kt][:, j*128:+128].T @ V[kt][:, dh*512:+512]
    then copy psum -> w_sb rows, store.
"""
import sys
import numpy as np

sys.path.insert(0, "/opt/trn_rl_repo")

import concourse.bass as bass  # noqa: E402
import concourse.bacc as bacc  # noqa: E402
import concourse.mybir as mybir  # noqa: E402
import concourse.tile as tile  # noqa: E402
from concourse.bass_utils import run_bass_kernel_spmd  # noqa: E402
from concourse.masks import make_identity  # noqa: E402
from contextlib import ExitStack  # noqa: E402

F32 = mybir.dt.float32
F32R = mybir.dt.float32r

NQ, NK, D = 4096, 4096, 1024
NCORES = 8
MQ = NQ // NCORES          # 512 query rows per core
NKT = NK // 128            # 32 key tiles
NDB = D // 128             # 8 d-blocks
NQT = MQ // 128            # 4 q tiles
KC = 2                     # K/V DMA chunk, in 128-row tiles (1 MB per chunk)
NCH = NK // (128 * KC)     # 16 chunks


def build_nc():
    nc = bacc.Bacc()
    q = nc.declare_dram_parameter("q", [MQ, D], F32R, isOutput=False)
    keys = nc.declare_dram_parameter("keys", [NK, D], F32R, isOutput=False)
    values = nc.declare_dram_parameter("values", [NK, D], F32R, isOutput=False)
    attn_out = nc.declare_dram_parameter("attn_out", [NK, MQ], F32R, isOutput=True)
    w_out = nc.declare_dram_parameter("w_out", [MQ, D], F32, isOutput=True)

    q_v = q.rearrange("(t p) d -> p t d", p=128)            # [128, 4, 1024]
    keys_v = keys.rearrange("(c t p) d -> c p t d", p=128, t=KC)
    vals_v = values.rearrange("(c t p) d -> c p t d", p=128, t=KC)
    w_v = w_out.rearrange("(j p) d -> p j d", p=128)        # [128, 4, 1024]

    with tile.TileContext(nc) as tc, ExitStack() as ctx:
        const_pool = ctx.enter_context(tc.tile_pool(name="const", bufs=1))
        qt_pool = ctx.enter_context(tc.tile_pool(name="qt", bufs=1))
        attn_pool = ctx.enter_context(tc.tile_pool(name="attn", bufs=1))
        kio_pool = ctx.enter_context(tc.tile_pool(name="kio", bufs=4))
        vio_pool = ctx.enter_context(tc.tile_pool(name="vio", bufs=4))
        work_pool = ctx.enter_context(tc.tile_pool(name="work", bufs=2))
        ktsb_pool = ctx.enter_context(tc.tile_pool(name="ktsb", bufs=3))
        small_pool = ctx.enter_context(tc.tile_pool(name="small", bufs=4))

        ident_f = const_pool.tile([128, 128], F32)
        make_identity(nc, ident_f[:])
        ident_r = const_pool.tile([128, 128], F32R)
        nc.vector.tensor_copy(ident_r[:], ident_f[:])

        QT = [qt_pool.tile([128, MQ], F32R, name=f"qt{db}") for db in range(NDB)]
        attn_sb = [attn_pool.tile([128, MQ], F32R, name=f"attn{kt}") for kt in range(NKT)]

        # prefetch first K chunks before anything else queues on DMA
        kc_tiles = {}
        for c in range(2):
            kc_tiles[c] = kio_pool.tile([128, KC, D], F32R, name=f"kc{c}", tag="kio")
            nc.sync.dma_start(kc_tiles[c][:], keys_v[c])

        # ---------------- Q prep ----------------
        with (
            tc.tile_pool(name="qprep", bufs=1) as qprep_pool,
            tc.tile_pool(name="qps", bufs=1) as qps_pool_sb,
            tc.tile_pool(name="qpsum", bufs=1, space="PSUM") as qps_pool,
        ):
            q_io = qprep_pool.tile([128, NQT, D], F32R)
            nc.sync.dma_start(q_io[:], q_v)
            qt_ps = [qps_pool.tile([128, MQ], F32R, name=f"qtps{db}")
                     for db in range(NDB)]
            for j in range(NQT):
                ssq = small_pool.tile([128, 1], F32, name=f"qssq{j}", tag="qssq")
                scratch = work_pool.tile([128, D], F32, name=f"qsc{j}", tag="qsc")
                nc.scalar.activation(scratch[:], q_io[:, j, :],
                                     mybir.ActivationFunctionType.Square,
                                     accum_out=ssq[:])
                rinv = small_pool.tile([128, 1], F32, name=f"qrinv{j}", tag="qrinv")
                nc.vector.reciprocal(rinv[:], ssq[:])
                nc.scalar.activation(rinv[:], rinv[:],
                                     mybir.ActivationFunctionType.Sqrt)
                qn_j = qprep_pool.tile([128, D], F32R, name=f"qn{j}")
                nc.vector.tensor_scalar_mul(qn_j[:], q_io[:, j, :], rinv[:])
                for db in range(NDB):
                    nc.tensor.transpose(qt_ps[db][:, j * 128:(j + 1) * 128],
                                        qn_j[:, db * 128:(db + 1) * 128],
                                        ident_r[:])
            for db in range(NDB):
                nc.vector.tensor_copy(QT[db][:], qt_ps[db][:])

        # ---------------- phase 1: attn tiles ----------------
        v_tiles = {}
        with (
            tc.tile_pool(name="ktps", bufs=2, space="PSUM") as ktps_pool,
            tc.tile_pool(name="aps", bufs=3, space="PSUM") as aps_pool,
        ):
            for c in range(NCH):
                if c in kc_tiles:
                    kc_t = kc_tiles[c]
                else:
                    kc_t = kio_pool.tile([128, KC, D], F32R, name=f"kc{c}", tag="kio")
                    nc.sync.dma_start(kc_t[:], keys_v[c])
                # trickle V prefetch during the second half of phase 1
                if c >= NCH - 4:
                    vc = c - (NCH - 4)
                    vt = vio_pool.tile([128, KC, D], F32R, name=f"vc{vc}", tag="vio")
                    nc.sync.dma_start(vt[:], vals_v[vc])
                    v_tiles[vc] = vt
                for t in range(KC):
                    kt = c * KC + t
                    k_tile = kc_t[:, t, :]
                    ssq = small_pool.tile([128, 1], F32, name=f"kssq{kt}", tag="kssq")
                    scratch = work_pool.tile([128, D], F32, name=f"ksc{kt}", tag="ksc")
                    nc.scalar.activation(scratch[:], k_tile,
                                         mybir.ActivationFunctionType.Square,
                                         accum_out=ssq[:])
                    rinv = small_pool.tile([128, 1], F32, name=f"krinv{kt}", tag="krinv")
                    nc.vector.reciprocal(rinv[:], ssq[:])
                    nc.scalar.activation(rinv[:], rinv[:],
                                         mybir.ActivationFunctionType.Sqrt)

                    kt_ps = ktps_pool.tile([128, D], F32R, name=f"ktps{kt}", tag="ktps")
                    for db in range(NDB):
                        nc.tensor.transpose(kt_ps[:, db * 128:(db + 1) * 128],
                                            k_tile[:, db * 128:(db + 1) * 128],
                                            ident_r[:])
                    kt_sb = ktsb_pool.tile([128, D], F32R, name=f"ktsb{kt}", tag="ktsb")
                    nc.vector.tensor_copy(kt_sb[:], kt_ps[:])

                    a_ps = aps_pool.tile([128, MQ], F32, name=f"aps{kt}", tag="aps")
                    for db in range(NDB):
                        nc.tensor.matmul(a_ps[:],
                                         kt_sb[:, db * 128:(db + 1) * 128],
                                         QT[db][:],
                                         start=(db == 0), stop=(db == NDB - 1))
                    # attn = S * (1/|k|) with cast fp32 -> fp32r, on ACT
                    nc.scalar.activation(attn_sb[kt][:], a_ps[:],
                                         mybir.ActivationFunctionType.Copy,
                                         scale=rinv[:])
                    nc.sync.dma_start(attn_out[kt * 128:(kt + 1) * 128, :],
                                      attn_sb[kt][:])

        # ---------------- phase 2: weighted, natural orientation ----------------
        wsb_pool = ctx.enter_context(tc.tile_pool(name="wsb", bufs=1))
        with tc.tile_pool(name="wps", bufs=1, space="PSUM") as wps_pool:
            w_ps = [[wps_pool.tile([128, 512], F32, name=f"wps{j}_{dh}")
                     for dh in range(2)] for j in range(NQT)]
            for c in range(NCH):
                if c < 4:
                    vc_t = v_tiles[c]
                else:
                    vc_t = vio_pool.tile([128, KC, D], F32R, name=f"vc{c}", tag="vio")
                    nc.sync.dma_start(vc_t[:], vals_v[c])
                for t in range(KC):
                    kt = c * KC + t
                    for j in range(NQT):
                        for dh in range(2):
                            nc.tensor.matmul(
                                w_ps[j][dh][:],
                                attn_sb[kt][:, j * 128:(j + 1) * 128],
                                vc_t[:, t, dh * 512:(dh + 1) * 512],
                                start=(kt == 0), stop=(kt == NKT - 1))
            w_sb = wsb_pool.tile([128, NQT, D], F32)
            for j in range(NQT):
                for dh in range(2):
                    nc.vector.tensor_copy(w_sb[:, j, dh * 512:(dh + 1) * 512],
                                          w_ps[j][dh][:])
            nc.sync.dma_start(w_v, w_sb[:])

    nc.finalize()
    return nc


_NC_CACHE = None


def _get_nc():
    global _NC_CACHE
    if _NC_CACHE is None:
        _NC_CACHE = build_nc()
    return _NC_CACHE


def kernel(queries: np.ndarray, keys: np.ndarray, values: np.ndarray):
    queries = np.ascontiguousarray(queries, dtype=np.float32)
    keys = np.ascontiguousarray(keys, dtype=np.float32)
    values = np.ascontiguousarray(values, dtype=np.float32)

    nc = _get_nc()
    in_maps = [
        {"q": queries[i * MQ:(i + 1) * MQ], "keys": keys, "values": values}
        for i in range(NCORES)
    ]
    res = run_bass_kernel_spmd(nc, in_maps, list(range(NCORES)))
    attn = np.concatenate([r["attn_out"] for r in res.results], axis=1)
    weighted = np.concatenate([r["w_out"] for r in res.results], axis=0)
    return (weighted.astype(np.float32, copy=False),
            attn.astype(np.float32, copy=False))


# revision 9
# speedup vs baseline: 1.0779x; 1.0362x over previous
"""Cosine-similarity attention on 8 Trainium2 NeuronCores.

reference:
    attn = l2norm(keys) @ l2norm(queries).T          # [Nk, Nq]
    weighted = attn.T @ values                        # [Nq, D]
    returns (weighted, attn)

Sharding: queries split row-wise across 8 cores (512 rows each); keys/values
replicated. Core i computes attn[:, 512i:512i+512] and weighted[512i:512i+512, :].

Per-core dataflow (Nq_shard=512, Nk=4096, D=1024), matmuls in fp32r:
  - Q: load (split DMAs for early start), row-ssq (ACT Square+accum),
    rinv=1/sqrt, scale rows (DVE), PE-transpose -> QT[d][128, 512].
  - per K tile kt: load raw, row-ssq, PE-transpose raw K -> KT, 8 accumulating
    matmuls psum_attn += KT[db].T @ QT[db], attn_sb[kt] = psum * rinv_k
    (scaled copy with f32r cast on ACT), store. PE stream software-pipelined:
    transposes of kt+1 are emitted before matmuls of kt so the DVE KT copy
    latency is hidden.
  - weighted (both operands natural layout):
    for kt: for j: for dh: psum_w[j][dh] += attn[kt][:, j128].T @ V[kt][:, dh512]
    copies and stores staggered per j across DVE/ACT.
"""
import sys
import numpy as np

sys.path.insert(0, "/opt/trn_rl_repo")

import concourse.bass as bass  # noqa: E402
import concourse.bacc as bacc  # noqa: E402
import concourse.mybir as mybir  # noqa: E402
import concourse.tile as tile  # noqa: E402
from concourse.bass_utils import run_bass_kernel_spmd  # noqa: E402
from concourse.masks import make_identity  # noqa: E402
from contextlib import ExitStack  # noqa: E402

F32 = mybir.dt.float32
F32R = mybir.dt.float32r
ACT_SQUARE = mybir.ActivationFunctionType.Square
ACT_SQRT = mybir.ActivationFunctionType.Sqrt
ACT_COPY = mybir.ActivationFunctionType.Copy

NQ, NK, D = 4096, 4096, 1024
NCORES = 8
MQ = NQ // NCORES          # 512 query rows per core
NKT = NK // 128            # 32 key tiles
NDB = D // 128             # 8 d-blocks
NQT = MQ // 128            # 4 q tiles
KC = 2                     # K/V DMA chunk, in 128-row tiles (1 MB per chunk)
NCH = NK // (128 * KC)     # 16 chunks


def build_nc():
    nc = bacc.Bacc()
    q = nc.declare_dram_parameter("q", [MQ, D], F32R, isOutput=False)
    keys = nc.declare_dram_parameter("keys", [NK, D], F32R, isOutput=False)
    values = nc.declare_dram_parameter("values", [NK, D], F32R, isOutput=False)
    attn_out = nc.declare_dram_parameter("attn_out", [NK, MQ], F32R, isOutput=True)
    w_out = nc.declare_dram_parameter("w_out", [MQ, D], F32, isOutput=True)

    q_v = q.rearrange("(t p) d -> p t d", p=128)            # [128, 4, 1024]
    keys_v = keys.rearrange("(c t p) d -> c p t d", p=128, t=KC)
    vals_v = values.rearrange("(c t p) d -> c p t d", p=128, t=KC)
    w_v = w_out.rearrange("(j p) d -> p j d", p=128)        # [128, 4, 1024]

    with tile.TileContext(nc) as tc, ExitStack() as ctx:
        const_pool = ctx.enter_context(tc.tile_pool(name="const", bufs=1))
        qt_pool = ctx.enter_context(tc.tile_pool(name="qt", bufs=1))
        attn_pool = ctx.enter_context(tc.tile_pool(name="attn", bufs=1))
        kio_pool = ctx.enter_context(tc.tile_pool(name="kio", bufs=3))
        vio_pool = ctx.enter_context(tc.tile_pool(name="vio", bufs=6))
        sq_pool = ctx.enter_context(tc.tile_pool(name="sq", bufs=2))
        ktsb_pool = ctx.enter_context(tc.tile_pool(name="ktsb", bufs=3))
        small_pool = ctx.enter_context(tc.tile_pool(name="small", bufs=4))

        ident_f = const_pool.tile([128, 128], F32)
        make_identity(nc, ident_f[:])
        ident_r = const_pool.tile([128, 128], F32R)
        nc.vector.tensor_copy(ident_r[:], ident_f[:])

        QT = [qt_pool.tile([128, MQ], F32R, name=f"qt{db}") for db in range(NDB)]
        attn_sb = [attn_pool.tile([128, MQ], F32R, name=f"attn{kt}") for kt in range(NKT)]

        # ---------------- Q prep (+ early K prefetch) ----------------
        v_tiles = {}
        kc_tiles = {}
        with (
            tc.tile_pool(name="qprep", bufs=2) as qprep_pool,
            tc.tile_pool(name="qnp", bufs=1) as qn_pool,
            tc.tile_pool(name="qpsum", bufs=1, space="PSUM") as qps_pool,
        ):
            # q halves first so the normalize chain starts ASAP
            q_half = []
            for h in range(2):
                qh = qprep_pool.tile([128, 2, D], F32R, name=f"qh{h}", tag="qh")
                nc.sync.dma_start(qh[:], q_v[:, 2 * h:2 * h + 2, :])
                q_half.append(qh)
            for c in range(3):
                kc_tiles[c] = kio_pool.tile([128, KC, D], F32R, name=f"kc{c}", tag="kio")
                nc.sync.dma_start(kc_tiles[c][:], keys_v[c])

            qt_ps = [qps_pool.tile([128, MQ], F32R, name=f"qtps{db}")
                     for db in range(NDB)]
            for j in range(NQT):
                q_j = q_half[j // 2][:, j % 2, :]
                ssq = small_pool.tile([128, 1], F32, name=f"qssq{j}", tag="qssq")
                scratch = sq_pool.tile([128, D], F32, name=f"qsc{j}", tag="sq")
                nc.scalar.activation(scratch[:], q_j, ACT_SQUARE, accum_out=ssq[:])
                rinv = small_pool.tile([128, 1], F32, name=f"qrinv{j}", tag="qrinv")
                nc.vector.reciprocal(rinv[:], ssq[:])
                nc.scalar.activation(rinv[:], rinv[:], ACT_SQRT)
                qn_j = qn_pool.tile([128, D], F32R, name=f"qn{j}")
                nc.vector.tensor_scalar_mul(qn_j[:], q_j, rinv[:])
                for db in range(NDB):
                    nc.tensor.transpose(qt_ps[db][:, j * 128:(j + 1) * 128],
                                        qn_j[:, db * 128:(db + 1) * 128],
                                        ident_r[:])
            for db in range(NDB):
                nc.vector.tensor_copy(QT[db][:], qt_ps[db][:])

        # ---------------- phase 1: attn tiles (PE software-pipelined) ------
        with (
            tc.tile_pool(name="ktps", bufs=2, space="PSUM") as ktps_pool,
            tc.tile_pool(name="aps", bufs=3, space="PSUM") as aps_pool,
        ):
            kt_sbs = {}
            rinvs = {}

            def k_front(kt):
                """DMA (chunk granularity), ssq, transposes, KT copy for kt."""
                c, t = divmod(kt, KC)
                if t == 0:
                    if c in kc_tiles:
                        kc_tiles[c] = kc_tiles[c]
                    else:
                        kc_tiles[c] = kio_pool.tile([128, KC, D], F32R,
                                                    name=f"kc{c}", tag="kio")
                        nc.sync.dma_start(kc_tiles[c][:], keys_v[c])
                    # V prefetch trickle near the end of phase 1
                    if c >= NCH - 6:
                        vc = c - (NCH - 6)
                        vt = vio_pool.tile([128, KC, D], F32R,
                                           name=f"vc{vc}", tag="vio")
                        nc.sync.dma_start(vt[:], vals_v[vc])
                        v_tiles[vc] = vt
                k_tile = kc_tiles[c][:, t, :]
                ssq = small_pool.tile([128, 1], F32, name=f"kssq{kt}", tag="kssq")
                scratch = sq_pool.tile([128, D], F32, name=f"ksc{kt}", tag="sq")
                nc.scalar.activation(scratch[:], k_tile, ACT_SQUARE, accum_out=ssq[:])
                rinv = small_pool.tile([128, 1], F32, name=f"krinv{kt}", tag="krinv")
                nc.vector.reciprocal(rinv[:], ssq[:])
                nc.scalar.activation(rinv[:], rinv[:], ACT_SQRT)
                rinvs[kt] = rinv

                kt_ps = ktps_pool.tile([128, D], F32R, name=f"ktps{kt}", tag="ktps")
                for db in range(NDB):
                    nc.tensor.transpose(kt_ps[:, db * 128:(db + 1) * 128],
                                        k_tile[:, db * 128:(db + 1) * 128],
                                        ident_r[:])
                kt_sb = ktsb_pool.tile([128, D], F32R, name=f"ktsb{kt}", tag="ktsb")
                nc.vector.tensor_copy(kt_sb[:], kt_ps[:])
                kt_sbs[kt] = kt_sb

            def k_back(kt):
                """matmuls + scaled copy + store for kt."""
                kt_sb = kt_sbs.pop(kt)
                a_ps = aps_pool.tile([128, MQ], F32, name=f"aps{kt}", tag="aps")
                for db in range(NDB):
                    nc.tensor.matmul(a_ps[:],
                                     kt_sb[:, db * 128:(db + 1) * 128],
                                     QT[db][:],
                                     start=(db == 0), stop=(db == NDB - 1))
                nc.scalar.activation(attn_sb[kt][:], a_ps[:], ACT_COPY,
                                     scale=rinvs.pop(kt)[:])
                nc.sync.dma_start(attn_out[kt * 128:(kt + 1) * 128, :],
                                  attn_sb[kt][:])

            k_front(0)
            for kt in range(NKT - 1):
                k_front(kt + 1)
                k_back(kt)
            k_back(NKT - 1)

        # ---------------- phase 2: weighted, natural orientation ----------
        wsb_pool = ctx.enter_context(tc.tile_pool(name="wsb", bufs=1))
        with tc.tile_pool(name="wps", bufs=1, space="PSUM") as wps_pool:
            w_ps = [[wps_pool.tile([128, 512], F32, name=f"wps{j}_{dh}")
                     for dh in range(2)] for j in range(NQT)]
            for c in range(NCH):
                if c in v_tiles:
                    vc_t = v_tiles[c]
                else:
                    vc_t = vio_pool.tile([128, KC, D], F32R, name=f"vc{c}", tag="vio")
                    nc.sync.dma_start(vc_t[:], vals_v[c])
                    v_tiles[c] = vc_t
                for t in range(KC):
                    kt = c * KC + t
                    for j in range(NQT):
                        for dh in range(2):
                            nc.tensor.matmul(
                                w_ps[j][dh][:],
                                attn_sb[kt][:, j * 128:(j + 1) * 128],
                                vc_t[:, t, dh * 512:(dh + 1) * 512],
                                start=(kt == 0), stop=(kt == NKT - 1))
            w_sb = wsb_pool.tile([128, NQT, D], F32)
            for j in range(NQT):
                # stagger the two copies across DVE and ACT, store per j
                nc.vector.tensor_copy(w_sb[:, j, 0:512], w_ps[j][0][:])
                nc.scalar.activation(w_sb[:, j, 512:1024], w_ps[j][1][:], ACT_COPY)
                nc.sync.dma_start(w_v[:, j, :], w_sb[:, j, :])

    nc.finalize()
    return nc


_NC_CACHE = None


def _get_nc():
    global _NC_CACHE
    if _NC_CACHE is None:
        _NC_CACHE = build_nc()
    return _NC_CACHE


def kernel(queries: np.ndarray, keys: np.ndarray, values: np.ndarray):
    queries = np.ascontiguousarray(queries, dtype=np.float32)
    keys = np.ascontiguousarray(keys, dtype=np.float32)
    values = np.ascontiguousarray(values, dtype=np.float32)

    nc = _get_nc()
    in_maps = [
        {"q": queries[i * MQ:(i + 1) * MQ], "keys": keys, "values": values}
        for i in range(NCORES)
    ]
    res = run_bass_kernel_spmd(nc, in_maps, list(range(NCORES)))
    attn = np.concatenate([r["attn_out"] for r in res.results], axis=1)
    weighted = np.concatenate([r["w_out"] for r in res.results], axis=0)
    return (weighted.astype(np.float32, copy=False),
            attn.astype(np.float32, copy=False))


# revision 10
# speedup vs baseline: 1.1275x; 1.0460x over previous
"""Cosine-similarity attention on 8 Trainium2 NeuronCores.

reference:
    attn = l2norm(keys) @ l2norm(queries).T          # [Nk, Nq]
    weighted = attn.T @ values                        # [Nq, D]
    returns (weighted, attn)

Sharding: queries split row-wise across 8 cores (512 rows each); keys/values
replicated. Core i computes attn[:, 512i:512i+512] and weighted[512i:512i+512, :].

The kernel is HBM-bandwidth bound (~40MB/core at ~350GB/s), so everything is
kept in fp16 on chip (fp32 PSUM accumulation): K/V/Q are cast fp32->fp16
during the DMA load, attention is stored to HBM as fp16 and upcast on the
host. Matmul/transpose operands fp16 -> full FWL weight loads, 1 cycle/row.

Per-core dataflow (Nq_shard=512, Nk=4096, D=1024):
  - Q: load halves, row-ssq (ACT Square+accum), rinv=1/sqrt (DVE recip + ACT
    sqrt), scale rows, PE-transpose -> QT[db][128, 512] d-major tiles.
  - per K tile kt: cast-load, row-ssq, PE-transpose raw K -> KT, 8
    accumulating matmuls psum += KT[db].T @ QT[db] -> S [128(k), 512(q)],
    attn_sb[kt] = psum * rinv_k (ACT scaled copy, fp32->fp16), store.
    PE stream software-pipelined: transposes of kt+1 precede matmuls of kt.
  - weighted, both operands natural layout:
    for kt: for j: for dh: psum_w[j][dh] += attn[kt][:, j128].T @ V[kt][:, dh512]
    then copy+store per j (DVE/ACT staggered).
"""
import sys
import numpy as np

sys.path.insert(0, "/opt/trn_rl_repo")

import concourse.bass as bass  # noqa: E402
import concourse.bacc as bacc  # noqa: E402
import concourse.mybir as mybir  # noqa: E402
import concourse.tile as tile  # noqa: E402
from concourse.bass_utils import run_bass_kernel_spmd  # noqa: E402
from concourse.masks import make_identity  # noqa: E402
from contextlib import ExitStack  # noqa: E402

F32 = mybir.dt.float32
F16 = mybir.dt.float16
ACT_SQUARE = mybir.ActivationFunctionType.Square
ACT_SQRT = mybir.ActivationFunctionType.Sqrt
ACT_COPY = mybir.ActivationFunctionType.Copy

NQ, NK, D = 4096, 4096, 1024
NCORES = 8
MQ = NQ // NCORES          # 512 query rows per core
NKT = NK // 128            # 32 key tiles
NDB = D // 128             # 8 d-blocks
NQT = MQ // 128            # 4 q tiles
KC = 2                     # K/V DMA chunk, in 128-row tiles
NCH = NK // (128 * KC)     # 16 chunks


def build_nc():
    nc = bacc.Bacc()
    q = nc.declare_dram_parameter("q", [MQ, D], F32, isOutput=False)
    keys = nc.declare_dram_parameter("keys", [NK, D], F32, isOutput=False)
    values = nc.declare_dram_parameter("values", [NK, D], F32, isOutput=False)
    attn_out = nc.declare_dram_parameter("attn_out", [NK, MQ], F16, isOutput=True)
    w_out = nc.declare_dram_parameter("w_out", [MQ, D], F32, isOutput=True)

    q_v = q.rearrange("(t p) d -> p t d", p=128)            # [128, 4, 1024]
    keys_v = keys.rearrange("(c t p) d -> c p t d", p=128, t=KC)
    vals_v = values.rearrange("(c t p) d -> c p t d", p=128, t=KC)
    w_v = w_out.rearrange("(j p) d -> p j d", p=128)        # [128, 4, 1024]

    with tile.TileContext(nc) as tc, ExitStack() as ctx:
        const_pool = ctx.enter_context(tc.tile_pool(name="const", bufs=1))
        qt_pool = ctx.enter_context(tc.tile_pool(name="qt", bufs=1))
        attn_pool = ctx.enter_context(tc.tile_pool(name="attn", bufs=1))
        kio_pool = ctx.enter_context(tc.tile_pool(name="kio", bufs=4))
        vio_pool = ctx.enter_context(tc.tile_pool(name="vio", bufs=8))
        sq_pool = ctx.enter_context(tc.tile_pool(name="sq", bufs=2))
        ktsb_pool = ctx.enter_context(tc.tile_pool(name="ktsb", bufs=3))
        small_pool = ctx.enter_context(tc.tile_pool(name="small", bufs=4))

        ident_h = const_pool.tile([128, 128], F16)
        make_identity(nc, ident_h[:])
        # preload ACT Square/Sqrt tables before any data arrives
        warm = const_pool.tile([128, 1], F32)
        nc.scalar.activation(warm[:], ident_h[:, 0:1], ACT_SQUARE)
        nc.scalar.activation(warm[:], warm[:], ACT_SQRT)

        QT = [qt_pool.tile([128, MQ], F16, name=f"qt{db}") for db in range(NDB)]
        attn_sb = [attn_pool.tile([128, MQ], F16, name=f"attn{kt}") for kt in range(NKT)]

        # ---------------- Q prep (+ early K prefetch) ----------------
        v_tiles = {}
        kc_tiles = {}
        with (
            tc.tile_pool(name="qprep", bufs=2) as qprep_pool,
            tc.tile_pool(name="qnp", bufs=1) as qn_pool,
            tc.tile_pool(name="qpsum", bufs=1, space="PSUM") as qps_pool,
        ):
            q_half = []
            for h in range(2):
                qh = qprep_pool.tile([128, 2, D], F16, name=f"qh{h}", tag="qh")
                nc.gpsimd.dma_start(qh[:], q_v[:, 2 * h:2 * h + 2, :])
                q_half.append(qh)
            for c in range(3):
                kc_tiles[c] = kio_pool.tile([128, KC, D], F16, name=f"kc{c}", tag="kio")
                nc.gpsimd.dma_start(kc_tiles[c][:], keys_v[c])

            qt_ps = [qps_pool.tile([128, MQ], F16, name=f"qtps{db}")
                     for db in range(NDB)]
            for j in range(NQT):
                q_j = q_half[j // 2][:, j % 2, :]
                ssq = small_pool.tile([128, 1], F32, name=f"qssq{j}", tag="qssq")
                scratch = sq_pool.tile([128, D], F32, name=f"qsc{j}", tag="sq")
                nc.scalar.activation(scratch[:], q_j, ACT_SQUARE, accum_out=ssq[:])
                rinv = small_pool.tile([128, 1], F32, name=f"qrinv{j}", tag="qrinv")
                nc.vector.reciprocal(rinv[:], ssq[:])
                nc.scalar.activation(rinv[:], rinv[:], ACT_SQRT)
                qn_j = qn_pool.tile([128, D], F16, name=f"qn{j}")
                nc.vector.tensor_scalar_mul(qn_j[:], q_j, rinv[:])
                for db in range(NDB):
                    nc.tensor.transpose(qt_ps[db][:, j * 128:(j + 1) * 128],
                                        qn_j[:, db * 128:(db + 1) * 128],
                                        ident_h[:])
            for db in range(NDB):
                nc.vector.tensor_copy(QT[db][:], qt_ps[db][:])

        # ---------------- phase 1: attn tiles (PE software-pipelined) ------
        with (
            tc.tile_pool(name="ktps", bufs=2, space="PSUM") as ktps_pool,
            tc.tile_pool(name="aps", bufs=3, space="PSUM") as aps_pool,
        ):
            kt_sbs = {}
            rinvs = {}

            def k_front(kt):
                """DMA (chunk granularity), ssq, transposes, KT copy for kt."""
                c, t = divmod(kt, KC)
                if t == 0:
                    if c not in kc_tiles:
                        kc_tiles[c] = kio_pool.tile([128, KC, D], F16,
                                                    name=f"kc{c}", tag="kio")
                        nc.gpsimd.dma_start(kc_tiles[c][:], keys_v[c])
                    # V prefetch trickle through the back half of phase 1
                    if c >= NCH - 8:
                        vc = c - (NCH - 8)
                        vt = vio_pool.tile([128, KC, D], F16,
                                           name=f"vc{vc}", tag="vio")
                        nc.gpsimd.dma_start(vt[:], vals_v[vc])
                        v_tiles[vc] = vt
                k_tile = kc_tiles[c][:, t, :]
                ssq = small_pool.tile([128, 1], F32, name=f"kssq{kt}", tag="kssq")
                scratch = sq_pool.tile([128, D], F32, name=f"ksc{kt}", tag="sq")
                nc.scalar.activation(scratch[:], k_tile, ACT_SQUARE, accum_out=ssq[:])
                rinv = small_pool.tile([128, 1], F32, name=f"krinv{kt}", tag="krinv")
                nc.vector.reciprocal(rinv[:], ssq[:])
                nc.scalar.activation(rinv[:], rinv[:], ACT_SQRT)
                rinvs[kt] = rinv

                kt_ps = ktps_pool.tile([128, D], F16, name=f"ktps{kt}", tag="ktps")
                for db in range(NDB):
                    nc.tensor.transpose(kt_ps[:, db * 128:(db + 1) * 128],
                                        k_tile[:, db * 128:(db + 1) * 128],
                                        ident_h[:])
                kt_sb = ktsb_pool.tile([128, D], F16, name=f"ktsb{kt}", tag="ktsb")
                nc.vector.tensor_copy(kt_sb[:], kt_ps[:])
                kt_sbs[kt] = kt_sb

            def k_back(kt):
                """matmuls + scaled copy + store for kt."""
                kt_sb = kt_sbs.pop(kt)
                a_ps = aps_pool.tile([128, MQ], F32, name=f"aps{kt}", tag="aps")
                for db in range(NDB):
                    nc.tensor.matmul(a_ps[:],
                                     kt_sb[:, db * 128:(db + 1) * 128],
                                     QT[db][:],
                                     start=(db == 0), stop=(db == NDB - 1))
                nc.scalar.activation(attn_sb[kt][:], a_ps[:], ACT_COPY,
                                     scale=rinvs.pop(kt)[:])
                nc.sync.dma_start(attn_out[kt * 128:(kt + 1) * 128, :],
                                  attn_sb[kt][:])

            k_front(0)
            for kt in range(NKT - 1):
                k_front(kt + 1)
                k_back(kt)
            k_back(NKT - 1)

        # ---------------- phase 2: weighted, natural orientation ----------
        wsb_pool = ctx.enter_context(tc.tile_pool(name="wsb", bufs=1))
        with tc.tile_pool(name="wps", bufs=1, space="PSUM") as wps_pool:
            w_ps = [[wps_pool.tile([128, 512], F32, name=f"wps{j}_{dh}")
                     for dh in range(2)] for j in range(NQT)]
            for c in range(NCH):
                if c in v_tiles:
                    vc_t = v_tiles[c]
                else:
                    vc_t = vio_pool.tile([128, KC, D], F16, name=f"vc{c}", tag="vio")
                    nc.gpsimd.dma_start(vc_t[:], vals_v[c])
                    v_tiles[c] = vc_t
                for t in range(KC):
                    kt = c * KC + t
                    for j in range(NQT):
                        for dh in range(2):
                            nc.tensor.matmul(
                                w_ps[j][dh][:],
                                attn_sb[kt][:, j * 128:(j + 1) * 128],
                                vc_t[:, t, dh * 512:(dh + 1) * 512],
                                start=(kt == 0), stop=(kt == NKT - 1))
            w_sb = wsb_pool.tile([128, NQT, D], F32)
            for j in range(NQT):
                nc.vector.tensor_copy(w_sb[:, j, 0:512], w_ps[j][0][:])
                nc.scalar.activation(w_sb[:, j, 512:1024], w_ps[j][1][:], ACT_COPY)
                nc.sync.dma_start(w_v[:, j, :], w_sb[:, j, :])

    nc.finalize()
    return nc


_NC_CACHE = None


def _get_nc():
    global _NC_CACHE
    if _NC_CACHE is None:
        _NC_CACHE = build_nc()
    return _NC_CACHE


def kernel(queries: np.ndarray, keys: np.ndarray, values: np.ndarray):
    queries = np.ascontiguousarray(queries, dtype=np.float32)
    keys = np.ascontiguousarray(keys, dtype=np.float32)
    values = np.ascontiguousarray(values, dtype=np.float32)

    nc = _get_nc()
    in_maps = [
        {"q": queries[i * MQ:(i + 1) * MQ], "keys": keys, "values": values}
        for i in range(NCORES)
    ]
    res = run_bass_kernel_spmd(nc, in_maps, list(range(NCORES)))
    attn = np.concatenate(
        [r["attn_out"].astype(np.float32) for r in res.results], axis=1)
    weighted = np.concatenate([r["w_out"] for r in res.results], axis=0)
    return (weighted.astype(np.float32, copy=False), attn)


# revision 14
# speedup vs baseline: 1.1491x; 1.0191x over previous
"""Cosine-similarity attention on 8 Trainium2 NeuronCores.

reference:
    attn = l2norm(keys) @ l2norm(queries).T          # [Nk, Nq]
    weighted = attn.T @ values                        # [Nq, D]
    returns (weighted, attn)

Sharding: queries split row-wise across 8 cores (512 rows each); keys/values
replicated. Core i computes attn[:, 512i:512i+512] and weighted[512i:512i+512, :].

The kernel is HBM-bandwidth bound (~40MB/core at ~350GB/s), so everything is
kept in fp16 on chip (fp32 PSUM accumulation): K/V/Q are cast fp32->fp16
during the DMA load, attention is stored to HBM as fp16 and upcast on the
host. Matmul/transpose operands fp16 -> full FWL weight loads, 1 cycle/row.

Per-core dataflow (Nq_shard=512, Nk=4096, D=1024):
  - Q: load halves, row-ssq (ACT Square+accum), rinv=1/sqrt (DVE recip + ACT
    sqrt), scale rows, PE-transpose -> QT[db][128, 512] d-major tiles.
  - per K tile kt: cast-load, row-ssq, PE-transpose raw K -> KT, 8
    accumulating matmuls psum += KT[db].T @ QT[db] -> S [128(k), 512(q)],
    attn_sb[kt] = psum * rinv_k (ACT scaled copy, fp32->fp16), store.
    PE stream software-pipelined: transposes of kt+1 precede matmuls of kt.
  - weighted, both operands natural layout:
    for kt: for j: for dh: psum_w[j][dh] += attn[kt][:, j128].T @ V[kt][:, dh512]
    then copy+store per j (DVE/ACT staggered).
"""
import sys
import numpy as np

sys.path.insert(0, "/opt/trn_rl_repo")

import concourse.bass as bass  # noqa: E402
import concourse.bacc as bacc  # noqa: E402
import concourse.mybir as mybir  # noqa: E402
import concourse.tile as tile  # noqa: E402
from concourse.bass_utils import run_bass_kernel_spmd  # noqa: E402
from concourse.masks import make_identity  # noqa: E402
from contextlib import ExitStack  # noqa: E402

F32 = mybir.dt.float32
F16 = mybir.dt.float16
ACT_SQUARE = mybir.ActivationFunctionType.Square
ACT_SQRT = mybir.ActivationFunctionType.Sqrt
ACT_COPY = mybir.ActivationFunctionType.Copy

NQ, NK, D = 4096, 4096, 1024
NCORES = 8
MQ = NQ // NCORES          # 512 query rows per core
NKT = NK // 128            # 32 key tiles
NDB = D // 128             # 8 d-blocks
NQT = MQ // 128            # 4 q tiles
KC = 2                     # K/V DMA chunk, in 128-row tiles
NCH = NK // (128 * KC)     # 16 chunks


def build_nc():
    nc = bacc.Bacc()
    q = nc.declare_dram_parameter("q", [MQ, D], F32, isOutput=False)
    keys = nc.declare_dram_parameter("keys", [NK, D], F32, isOutput=False)
    values = nc.declare_dram_parameter("values", [NK, D], F32, isOutput=False)
    attn_out = nc.declare_dram_parameter("attn_out", [NK, MQ], F16, isOutput=True)
    w_out = nc.declare_dram_parameter("w_out", [MQ, D], F32, isOutput=True)

    q_v = q.rearrange("(t p) d -> p t d", p=128)            # [128, 4, 1024]
    keys_v = keys.rearrange("(c t p) d -> c p t d", p=128, t=KC)
    vals_v = values.rearrange("(c t p) d -> c p t d", p=128, t=KC)
    w_v = w_out.rearrange("(j p) d -> p j d", p=128)        # [128, 4, 1024]

    with tile.TileContext(nc) as tc, ExitStack() as ctx:
        const_pool = ctx.enter_context(tc.tile_pool(name="const", bufs=1))
        qt_pool = ctx.enter_context(tc.tile_pool(name="qt", bufs=1))
        attn_pool = ctx.enter_context(tc.tile_pool(name="attn", bufs=1))
        kio_pool = ctx.enter_context(tc.tile_pool(name="kio", bufs=6))
        vio_pool = ctx.enter_context(tc.tile_pool(name="vio", bufs=8))
        sq_pool = ctx.enter_context(tc.tile_pool(name="sq", bufs=2))
        ktsb_pool = ctx.enter_context(tc.tile_pool(name="ktsb", bufs=3))
        small_pool = ctx.enter_context(tc.tile_pool(name="small", bufs=4))

        ident_h = const_pool.tile([128, 128], F16)
        make_identity(nc, ident_h[:])
        # preload ACT Square/Sqrt tables before any data arrives
        warm = const_pool.tile([128, 1], F32)
        nc.scalar.activation(warm[:], ident_h[:, 0:1], ACT_SQUARE)
        nc.scalar.activation(warm[:], warm[:], ACT_SQRT)

        QT = [qt_pool.tile([128, MQ], F16, name=f"qt{db}") for db in range(NDB)]
        attn_sb = [attn_pool.tile([128, MQ], F16, name=f"attn{kt}") for kt in range(NKT)]

        # ---------------- Q prep (+ early K prefetch) ----------------
        v_tiles = {}
        kc_tiles = {}
        with (
            tc.tile_pool(name="qprep", bufs=2) as qprep_pool,
            tc.tile_pool(name="qnp", bufs=1) as qn_pool,
            tc.tile_pool(name="qpsum", bufs=1, space="PSUM") as qps_pool,
        ):
            q_half = []
            for h in range(2):
                qh = qprep_pool.tile([128, 2, D], F16, name=f"qh{h}", tag="qh")
                nc.gpsimd.dma_start(qh[:], q_v[:, 2 * h:2 * h + 2, :])
                q_half.append(qh)
            for c in range(3):
                kc_tiles[c] = kio_pool.tile([128, KC, D], F16, name=f"kc{c}", tag="kio")
                nc.gpsimd.dma_start(kc_tiles[c][:], keys_v[c])

            qt_ps = [qps_pool.tile([128, MQ], F16, name=f"qtps{db}")
                     for db in range(NDB)]
            for j in range(NQT):
                q_j = q_half[j // 2][:, j % 2, :]
                ssq = small_pool.tile([128, 1], F32, name=f"qssq{j}", tag="qssq")
                scratch = sq_pool.tile([128, D], F32, name=f"qsc{j}", tag="sq")
                nc.scalar.activation(scratch[:], q_j, ACT_SQUARE, accum_out=ssq[:])
                rinv = small_pool.tile([128, 1], F32, name=f"qrinv{j}", tag="qrinv")
                nc.vector.reciprocal(rinv[:], ssq[:])
                nc.scalar.activation(rinv[:], rinv[:], ACT_SQRT)
                qn_j = qn_pool.tile([128, D], F16, name=f"qn{j}")
                nc.vector.tensor_scalar_mul(qn_j[:], q_j, rinv[:])
                for db in range(NDB):
                    nc.tensor.transpose(qt_ps[db][:, j * 128:(j + 1) * 128],
                                        qn_j[:, db * 128:(db + 1) * 128],
                                        ident_h[:])
            for db in range(NDB):
                nc.vector.tensor_copy(QT[db][:], qt_ps[db][:])

        # ---------------- phase 1: attn tiles (PE software-pipelined) ------
        with (
            tc.tile_pool(name="ktps", bufs=2, space="PSUM") as ktps_pool,
            tc.tile_pool(name="aps", bufs=3, space="PSUM") as aps_pool,
        ):
            kt_sbs = {}
            rinvs = {}

            def k_front(kt):
                """DMA (chunk granularity), ssq, transposes, KT copy for kt."""
                c, t = divmod(kt, KC)
                if t == 0:
                    if c not in kc_tiles:
                        kc_tiles[c] = kio_pool.tile([128, KC, D], F16,
                                                    name=f"kc{c}", tag="kio")
                        nc.gpsimd.dma_start(kc_tiles[c][:], keys_v[c])
                k_tile = kc_tiles[c][:, t, :]
                ssq = small_pool.tile([128, 1], F32, name=f"kssq{kt}", tag="kssq")
                scratch = sq_pool.tile([128, D], F32, name=f"ksc{kt}", tag="sq")
                nc.scalar.activation(scratch[:], k_tile, ACT_SQUARE, accum_out=ssq[:])
                rinv = small_pool.tile([128, 1], F32, name=f"krinv{kt}", tag="krinv")
                nc.vector.reciprocal(rinv[:], ssq[:])
                nc.scalar.activation(rinv[:], rinv[:], ACT_SQRT)
                rinvs[kt] = rinv

                kt_ps = ktps_pool.tile([128, D], F16, name=f"ktps{kt}", tag="ktps")
                for db in range(NDB):
                    nc.tensor.transpose(kt_ps[:, db * 128:(db + 1) * 128],
                                        k_tile[:, db * 128:(db + 1) * 128],
                                        ident_h[:])
                kt_sb = ktsb_pool.tile([128, D], F16, name=f"ktsb{kt}", tag="ktsb")
                nc.vector.tensor_copy(kt_sb[:], kt_ps[:])
                kt_sbs[kt] = kt_sb

            def k_back(kt):
                """matmuls + scaled copy + store for kt."""
                kt_sb = kt_sbs.pop(kt)
                a_ps = aps_pool.tile([128, MQ], F32, name=f"aps{kt}", tag="aps")
                for db in range(NDB):
                    nc.tensor.matmul(a_ps[:],
                                     kt_sb[:, db * 128:(db + 1) * 128],
                                     QT[db][:],
                                     start=(db == 0), stop=(db == NDB - 1))
                nc.scalar.activation(attn_sb[kt][:], a_ps[:], ACT_COPY,
                                     scale=rinvs.pop(kt)[:])
                nc.sync.dma_start(attn_out[kt * 128:(kt + 1) * 128, :],
                                  attn_sb[kt][:])

            k_front(0)
            for kt in range(NKT - 1):
                k_front(kt + 1)
                k_back(kt)
                if kt == NKT - 2:
                    # all K DMAs are queued now; queue the first V chunks
                    # behind them on the SWDGE ring
                    for vc in range(8):
                        vt = vio_pool.tile([128, KC, D], F16,
                                           name=f"vc{vc}", tag="vio")
                        nc.gpsimd.dma_start(vt[:], vals_v[vc])
                        v_tiles[vc] = vt
            k_back(NKT - 1)

        # ---------------- phase 2: weighted, natural orientation ----------
        wsb_pool = ctx.enter_context(tc.tile_pool(name="wsb", bufs=1))
        with tc.tile_pool(name="wps", bufs=1, space="PSUM") as wps_pool:
            w_ps = [[wps_pool.tile([128, 512], F32, name=f"wps{j}_{dh}")
                     for dh in range(2)] for j in range(NQT)]
            for c in range(NCH):
                if c in v_tiles:
                    vc_t = v_tiles[c]
                else:
                    vc_t = vio_pool.tile([128, KC, D], F16, name=f"vc{c}", tag="vio")
                    nc.gpsimd.dma_start(vc_t[:], vals_v[c])
                    v_tiles[c] = vc_t
                for t in range(KC):
                    kt = c * KC + t
                    for j in range(NQT):
                        for dh in range(2):
                            nc.tensor.matmul(
                                w_ps[j][dh][:],
                                attn_sb[kt][:, j * 128:(j + 1) * 128],
                                vc_t[:, t, dh * 512:(dh + 1) * 512],
                                start=(kt == 0), stop=(kt == NKT - 1))
            w_sb = wsb_pool.tile([128, NQT, D], F32)
            for j in range(NQT):
                nc.vector.tensor_copy(w_sb[:, j, 0:512], w_ps[j][0][:])
                nc.scalar.activation(w_sb[:, j, 512:1024], w_ps[j][1][:], ACT_COPY)
                nc.sync.dma_start(w_v[:, j, :], w_sb[:, j, :])

    nc.finalize()
    return nc


_NC_CACHE = None


def _get_nc():
    global _NC_CACHE
    if _NC_CACHE is None:
        _NC_CACHE = build_nc()
    return _NC_CACHE


def kernel(queries: np.ndarray, keys: np.ndarray, values: np.ndarray):
    queries = np.ascontiguousarray(queries, dtype=np.float32)
    keys = np.ascontiguousarray(keys, dtype=np.float32)
    values = np.ascontiguousarray(values, dtype=np.float32)

    nc = _get_nc()
    in_maps = [
        {"q": queries[i * MQ:(i + 1) * MQ], "keys": keys, "values": values}
        for i in range(NCORES)
    ]
    res = run_bass_kernel_spmd(nc, in_maps, list(range(NCORES)))
    attn = np.concatenate(
        [r["attn_out"].astype(np.float32) for r in res.results], axis=1)
    weighted = np.concatenate([r["w_out"] for r in res.results], axis=0)
    return (weighted.astype(np.float32, copy=False), attn)
